# revision 19
# baseline (speedup 1.0000x reference)
# Trainium2 Bass kernel for CausalSelfAttention (B=2, T=2048, C=1024, H=16, D=64)
# with periodic mask: causal AND (key_col % 4 != 3).
#
# Sharding (8 NeuronCores): core c = (b, g) with b = c//4 (batch), g = c%4
# (head group of 4 heads). Each core computes QKV for its 4 heads, attention,
# and a partial output projection y_heads @ Wp[rows] in bf16. Host sums the 4
# partials per batch in fp32 and adds bp (tensor-parallel reduce).
#
# Key device-side choices:
#  - x arrives pre-transposed per batch (xT = x[b].T, [C, T]) so the
#    contraction dim C sits on SBUF partitions for all QKV matmuls.
#  - The periodic mask is exploited as compaction: keys at t%4==3 are never
#    attended; K^T is computed directly at the 1536 kept positions (the K
#    matmul's moving operand skips dropped columns), V is compacted with
#    0/1-selection matmuls.
#  - Scores are produced transposed (S^T[tk_kept, tq]) so softmax-normalized
#    probabilities feed the AV matmul directly as the moving operand; head
#    pairs co-execute on the PE via row groups 0/64.
#  - Boundary (causal-frontier) tiles skip the fully-masked column range both
#    in the scores/exp and in the AV accumulation (no memsets needed).
#  - Softmax row sums ride along the AV matmul via a 64-wide ones block in
#    the stationary operand; reciprocal = exp(-ln(s)) on ScalarE, batched per
#    (head-pair, window) over both heads at once.
#  - Everything runs bf16 on the PE (fp32 PSUM accumulate): fp32 matmuls
#    would run at half rate (fp32_mode=HIGH).
#  - ~40 warm-up matmuls at t=0 lift the PE HAM clock gate from 1.2 GHz to
#    2.4 GHz while the first input DMAs are in flight.
#  - Emission interleaves QKV(j+1) and outproj(j-1) work items between
#    attention chunk-visits of window j so the PE stays fed while ScalarE
#    works through the exp stream.

import ml_dtypes
import numpy as np

B, T, C, H, D = 2, 2048, 1024, 16, 64
HG = 4          # heads per core
CG = HG * D     # = 256 columns of C per core
TK = (T // 4) * 3   # 1536 kept key positions
NTK = TK // 128     # 12 kept-key chunks of 128
SCALE = 1.0 / 8.0   # 1/sqrt(D)
NWARM = 10          # PE warm-up matmuls

_CACHE = {}


def _split_multi_waits(nc, mybir):
    # The pinned walrus here encodes at most 1 sync-wait per instruction
    # (2 for EventSemaphore). Hoist excess waits onto standalone NoOps that
    # precede the instruction on the same engine.
    f = nc.m.functions[0]
    n = 0
    for b in f.blocks:
        insts = list(b.instructions)
        out = []
        changed = False
        for inst in insts:
            si = inst.sync_info
            if si is not None:
                waits = list(si.on_wait)
                cap = 2 if isinstance(inst, mybir.InstEventSemaphore) else 1
                if len(waits) > cap:
                    for w in waits[cap:]:
                        out.append(mybir.InstNoOp(
                            name=f"{inst.name}-ws{n}", engine=inst.engine,
                            ins=[], outs=[],
                            sync_info=mybir.SyncInfo(on_wait=[w], on_update=[])))
                        n += 1
                    inst.sync_info = mybir.SyncInfo(
                        on_wait=waits[:cap], on_update=list(si.on_update))
                    changed = True
            out.append(inst)
        if changed:
            b.instructions = out
    return n


def _build_bass(split=True):
    import concourse.bass as bass
    import concourse.tile as tile
    import concourse.mybir as mybir

    f32 = mybir.dt.float32
    bf16 = mybir.dt.bfloat16

    nc = bass.Bass("TRN2", debug=False, num_devices=8)

    xt_d = nc.dram_tensor("xt", [C, T], bf16, kind="ExternalInput").ap()
    wq_d = nc.dram_tensor("wq", [C, CG], bf16, kind="ExternalInput").ap()
    wk_d = nc.dram_tensor("wk", [C, CG], bf16, kind="ExternalInput").ap()
    wv_d = nc.dram_tensor("wv", [C, CG], bf16, kind="ExternalInput").ap()
    wp_d = nc.dram_tensor("wp", [CG, C], bf16, kind="ExternalInput").ap()
    bq_d = nc.dram_tensor("bq2", [128, 2], f32, kind="ExternalInput").ap()
    bk_d = nc.dram_tensor("bk2", [128, 2], f32, kind="ExternalInput").ap()
    bvb_d = nc.dram_tensor("bvb", [128, HG, D], f32, kind="ExternalInput").ap()
    cm_d = nc.dram_tensor("cmask", [128, 3, 512], bf16, kind="ExternalInput").ap()
    gs_d = nc.dram_tensor("gsel", [128, 6, 128], bf16, kind="ExternalInput").ap()
    out_d = nc.dram_tensor("out", [T, C], bf16, kind="ExternalOutput").ap()

    Exp = mybir.ActivationFunctionType.Exp
    Ln = mybir.ActivationFunctionType.Ln
    MULT = mybir.AluOpType.mult

    with tile.TileContext(nc) as tc, \
         tc.tile_pool(name="persist", bufs=1) as persist, \
         tc.tile_pool(name="work", bufs=1) as work, \
         tc.tile_pool(name="ps_mm", space="PSUM", bufs=2) as ps_mm, \
         tc.tile_pool(name="ps_y", space="PSUM", bufs=2) as ps_y:
        # ps_mm: 2 slots x 2 banks shared by scores tiles and the transient
        # QKV/outproj accumulators; ps_y: double-buffered AV accumulators so
        # head-pair rows overlap across the normalize chain. 4+4 = 8 banks.
        ps_acc = ps_mm
        ps_sc = ps_mm
        # ---------- persistent SBUF ----------
        qt = [persist.tile([128, T], bf16, name=f"qt{m}", tag=f"qt{m}") for m in range(2)]
        kt = [persist.tile([128, TK], bf16, name=f"kt{m}", tag=f"kt{m}") for m in range(2)]
        vsb = persist.tile([128, NTK, HG, 2 * D], bf16, name="vsb", tag="vsb")
        yt = [persist.tile([128, T], bf16, name=f"yt{m}", tag=f"yt{m}") for m in range(2)]
        cmask = persist.tile([128, 3, 512], bf16, name="cmask", tag="cmask")
        bqs = persist.tile([128, 2], f32, name="bqs", tag="bqs")
        bks = persist.tile([128, 2], f32, name="bks", tag="bks")
        bvb = persist.tile([128, HG, D], f32, name="bvb", tag="bvb")
        bvf = bvb[:].rearrange("p h d -> p (h d)")
        wp_t = persist.tile([128, 2, C], bf16, name="wp_t", tag="wp_t")
        gsel = persist.tile([128, 6, 128], bf16, name="gsel", tag="gsel")
        wq_t = persist.tile([128, 8, CG], bf16, name="wq_t", tag="wq_t")
        wk_t = persist.tile([128, 8, CG], bf16, name="wk_t", tag="wk_t")
        wv_t = persist.tile([128, 8, CG], bf16, name="wv_t", tag="wv_t")
        warm = persist.tile([128, 512], bf16, name="warm", tag="warm")

        # ---------- DMA plan ----------
        # gpsimd: memsets first (warm-up + ones block), then SWDGE for
        # constants and wp (keeps both HWDGE rings free for wq/wk/wv/x).
        nc.gpsimd.memset(warm[:], 0.0)
        nc.gpsimd.memset(vsb[:, :, :, D:2 * D], 1.0)
        nc.gpsimd.dma_start(bqs[:], bq_d[:])
        nc.gpsimd.dma_start(bks[:], bk_d[:])
        nc.gpsimd.dma_start(bvb[:], bvb_d[:])
        nc.gpsimd.dma_start(wp_t[:], wp_d.rearrange("(k p) n -> p k n", p=128))

        xr = xt_d.rearrange("(k p) t -> p k t", p=128)
        xt = []   # [j] -> [128, 8, 512] column window (k on the free axis)
        for j in range(4):
            xw = work.tile([128, 8, 512], bf16, name=f"x{j}", tag=f"xtw{j % 2}",
                           bufs=2)
            xt.append(xw)

        # sync HWDGE ring: wq halves and first x half interleaved so the
        # first QKV accumulation chain can start as early as possible.
        wqr = wq_d.rearrange("(k p) n -> p k n", p=128)
        nc.sync.dma_start(wq_t[:, 0:4, :], wqr[:, 0:4, :])
        nc.sync.dma_start(xt[0][:, 0:4, :], xr[:, 0:4, 0:512])
        nc.sync.dma_start(wq_t[:, 4:8, :], wqr[:, 4:8, :])
        # scalar HWDGE ring: second half of window 0 first, then wk/wv +
        # small bf16 constants. Scalar engine is idle until the first exp.
        nc.scalar.dma_start(xt[0][:, 4:8, :], xr[:, 4:8, 0:512])
        nc.scalar.dma_start(wk_t[:], wk_d.rearrange("(k p) n -> p k n", p=128))
        nc.scalar.dma_start(wv_t[:, 0:4, :],
                            wv_d.rearrange("(k p) n -> p k n", p=128)[:, 0:4, :])
        nc.scalar.dma_start(wv_t[:, 4:8, :],
                            wv_d.rearrange("(k p) n -> p k n", p=128)[:, 4:8, :])
        nc.scalar.dma_start(cmask[:], cm_d[:])
        nc.scalar.dma_start(gsel[:], gs_d[:])
        # remaining x windows on the sync ring
        for j in range(1, 4):
            for k4 in range(2):
                nc.sync.dma_start(xt[j][:, 4 * k4:4 * k4 + 4, :],
                                  xr[:, 4 * k4:4 * k4 + 4, 512 * j:512 * (j + 1)])

        # ---------- PE warm-up (lift HAM to 2.4 GHz during input DMA) ----------
        pw = ps_sc.tile([128, 2, 512], f32, tag="mm")
        for _ in range(NWARM):
            nc.tensor.matmul(pw[:, 0, :], warm[:, 0:128], warm[:],
                             start=True, stop=True)

        # ---------- work items ----------
        def q_item(j, m):
            def emit():
                pq = ps_acc.tile([128, 512], f32, tag="mm")
                for k in range(8):
                    nc.tensor.matmul(pq[:], wq_t[:, k, 128 * m:128 * (m + 1)],
                                     xt[j][:, k, :], start=(k == 0), stop=(k == 7))
                nc.vector.tensor_scalar_add(qt[m][:, 512 * j:512 * (j + 1)],
                                            pq[:], bqs[:, m:m + 1])
            return emit

        def k_item(j, m):
            def emit():
                pk = ps_acc.tile([128, 512], f32, tag="mm")
                for k in range(8):
                    nc.tensor.matmul(pk[:], wk_t[:, k, 128 * m:128 * (m + 1)],
                                     xt[j][:, k, :], start=(k == 0), stop=(k == 7))
                # compact to kept key columns (drop t%4==3) during the copy
                pkc = pk[:].rearrange("p (a b) -> p a b", b=4)[:, :, 0:3]
                nc.vector.tensor_scalar_add(kt[m][:, 384 * j:384 * (j + 1)],
                                            pkc, bks[:, m:m + 1])
            return emit

        vfull = {}
        def v_item(j, mm):
            def emit():
                pv = ps_acc.tile([128, 512], f32, tag="mm")
                for k in range(8):
                    nc.tensor.matmul(pv[:, 0:CG],
                                     xt[j][:, k, 128 * mm:128 * (mm + 1)],
                                     wv_t[:, k, :], start=(k == 0), stop=(k == 7))
                vf = work.tile([128, CG], bf16, name=f"vf{j}_{mm}", tag="vf",
                               bufs=6)
                nc.vector.scalar_tensor_tensor(
                    out=vf[:], in0=pv[:, 0:CG], scalar=1.0, in1=bvf[:],
                    op0=mybir.AluOpType.bypass, op1=mybir.AluOpType.add)
                vfull[(j, mm)] = vf
            return emit

        def g_item(j, s):
            def emit():
                i = 3 * j + s
                pvk = ps_acc.tile([128, 512], f32, tag="mm")
                nc.tensor.matmul(pvk[:, 0:CG], gsel[:, 2 * s, :],
                                 vfull[(j, s)][:], start=True, stop=False)
                nc.tensor.matmul(pvk[:, 0:CG], gsel[:, 2 * s + 1, :],
                                 vfull[(j, s + 1)][:], start=False, stop=True)
                nc.vector.tensor_copy(
                    vsb[:, i, :, 0:D],
                    pvk[:, 0:CG].rearrange("p (h d) -> p h d", d=D))
            return emit

        def qkv_items(j):
            # gathers interleaved behind the V chains they depend on, so the
            # short gather matmul pairs overlap the long V accumulations
            return ([q_item(j, m) for m in range(2)]
                    + [k_item(j, m) for m in range(2)]
                    + [v_item(j, 0), v_item(j, 1), v_item(j, 2), g_item(j, 0),
                       v_item(j, 3), g_item(j, 1), g_item(j, 2)])

        def outproj_item(m):
            def emit():
                stage = work.tile([128, C], bf16, tag="stage", bufs=2)
                for n in range(2):
                    po = ps_acc.tile([128, 512], f32, tag="mm")
                    for k2 in range(2):
                        nc.tensor.matmul(
                            po[:], yt[k2][:, 128 * m:128 * (m + 1)],
                            wp_t[:, k2, 512 * n:512 * (n + 1)],
                            start=(k2 == 0), stop=(k2 == 1))
                    nc.vector.tensor_copy(stage[:, 512 * n:512 * (n + 1)], po[:])
                nc.sync.dma_start(out_d[128 * m:128 * (m + 1), :], stage[:])
            return emit

        # ---------- QKV window 0 (nothing to interleave with) ----------
        for it in qkv_items(0):
            it()

        # ---------- attention windows with interleaved fillers ----------
        for j in range(4):
            jwin = slice(512 * j, 512 * (j + 1))
            ntile = 3 * (j + 1)
            nb0 = ntile - 3  # first boundary tile index
            fillers = []
            if j < 3:
                fillers += qkv_items(j + 1)
            if j >= 1:
                fillers += [outproj_item(m) for m in range(4 * (j - 1), 4 * j)]
            nvisit = 2 * ntile
            nfill = len(fillers)
            vdone = 0
            fdone = 0

            for hp in range(2):
                pys = ps_y.tile([128, 2, 512], f32, tag="py")
                for i in range(ntile):
                    u = i - nb0
                    # boundary tiles u=1,2: cols [0:off) are fully masked —
                    # skipped in scores, exp, and the AV accumulation.
                    off = (0, 128, 320)[u] if u >= 1 else 0
                    ps2 = ps_sc.tile([128, 2, 512], f32, tag="mm")
                    pt2 = work.tile([128, 2, 512], bf16, tag="pt2", bufs=4)
                    for q in range(2):  # q: row group (head 2*hp + q)
                        nc.tensor.matmul(
                            ps2[:, q, off:512],
                            kt[hp][64 * q:64 * q + 64, 128 * i:128 * (i + 1)],
                            qt[hp][64 * q:64 * q + 64,
                                   512 * j + off:512 * (j + 1)],
                            start=True, stop=True)
                    nc.scalar.activation(pt2[:, :, off:512], ps2[:, :, off:512],
                                         Exp, bias=0.0, scale=SCALE)
                    if u >= 0:  # boundary tile: causal mask (both heads)
                        w = (192, 384, 512)[u]
                        for q in range(2):
                            nc.vector.tensor_tensor(
                                pt2[:, q, off:w], pt2[:, q, off:w],
                                cmask[:, u, off:w], op=MULT)
                    for q in range(2):
                        nc.tensor.matmul(
                            pys[:, q, off:512], vsb[:, i, 2 * hp + q, :],
                            pt2[:, q, off:512],
                            start=(i == 0), stop=(i == ntile - 1))
                    # interleave filler work to keep the PE fed while the
                    # scalar engine chews through the exp stream
                    vdone += 1
                    while fillers and fdone < (nfill * vdone) // nvisit:
                        fillers.pop(0)()
                        fdone += 1

                # a filler around the row boundary covers the normalize chain
                if fillers:
                    fillers.pop(0)()
                    fdone += 1
                # softmax normalization for both heads of this pair at once:
                # rec = exp(-ln(rowsum)), rowsums replicated at rows 64:128
                lns = work.tile([64, 2, 512], f32, tag="lns", bufs=2)
                rec = work.tile([64, 2, 512], f32, tag="rec", bufs=2)
                nc.scalar.activation(lns[:], pys[64:128, :, :], Ln)
                nc.scalar.activation(rec[:], lns[:], Exp, bias=0.0, scale=-1.0)
                for q in range(2):
                    nc.vector.tensor_tensor(
                        yt[hp][64 * q:64 * q + 64, jwin],
                        pys[0:64, q, :], rec[:, q, :], op=MULT)

            # any leftover fillers for this window
            for it in fillers:
                it()

        # ---------- output projection for the last window ----------
        for m in range(12, 16):
            outproj_item(m)()

    if split:
        _split_multi_waits(nc, mybir)
    return nc


def _get_nc():
    if "nc" not in _CACHE:
        _CACHE["nc"] = _build_bass()
    return _CACHE["nc"]


def _host_maps(inputs):
    x = np.asarray(inputs["x"], np.float32)
    Wq = np.asarray(inputs["Wq"], np.float32)
    Wk = np.asarray(inputs["Wk"], np.float32)
    Wv = np.asarray(inputs["Wv"], np.float32)
    Wp = np.asarray(inputs["Wp"], np.float32)
    bq = np.asarray(inputs["bq"], np.float32)
    bk = np.asarray(inputs["bk"], np.float32)
    bv = np.asarray(inputs["bv"], np.float32)

    # causal masks in compacted key coordinates: 3 boundary chunks
    p = np.arange(128)
    f = np.arange(512)
    cm = np.zeros((128, 3, 512), np.float32)
    for u in range(3):
        q = 128 * u + p
        g = (q // 3) * 4 + (q % 3)
        cm[:, u, :] = (f[None, :] >= g[:, None]).astype(np.float32)

    # V row-gather selection matrices: kept chunk i = 3k+s draws rows from
    # original chunks 4k+s and 4k+s+1; G[s][side][p, m] = 1 iff kept row m
    # maps to row p of that original chunk.
    gs = np.zeros((128, 6, 128), np.float32)
    for s in range(3):
        for m in range(128):
            orr = ((128 * s + m) // 3) * 4 + (128 * s + m) % 3
            side = 0 if orr < 128 * (s + 1) else 1
            gs[orr - 128 * (s + side), 2 * s + side, m] = 1.0

    xts = [np.ascontiguousarray(x[b].T).astype(ml_dtypes.bfloat16) for b in range(B)]
    maps = []
    for c in range(8):
        b, g = c // 4, c % 4
        sl = slice(CG * g, CG * (g + 1))
        maps.append({
            "xt": xts[b],
            "wq": np.ascontiguousarray(Wq[:, sl]).astype(ml_dtypes.bfloat16),
            "wk": np.ascontiguousarray(Wk[:, sl]).astype(ml_dtypes.bfloat16),
            "wv": np.ascontiguousarray(Wv[:, sl]).astype(ml_dtypes.bfloat16),
            "wp": np.ascontiguousarray(Wp[sl, :]).astype(ml_dtypes.bfloat16),
            "bq2": np.ascontiguousarray(bq[sl].reshape(2, 128).T),
            "bk2": np.ascontiguousarray(bk[sl].reshape(2, 128).T),
            "bvb": np.ascontiguousarray(
                np.broadcast_to(bv[sl].reshape(HG, D), (128, HG, D))),
            "cmask": cm.astype(ml_dtypes.bfloat16),
            "gsel": gs.astype(ml_dtypes.bfloat16),
        })
    return maps


def _combine(results, inputs):
    bp = np.asarray(inputs["bp"], np.float32)
    out = np.zeros((B, T, C), np.float32)
    for c in range(8):
        out[c // 4] += np.asarray(results[c]["out"], dtype=np.float32)
    out += bp[None, None, :]
    return out


def _run(inputs, profile_dir=None, trace_cores=None):
    nc = _get_nc()
    maps = _host_maps(inputs)
    from concourse.bass_utils import run_bass_kernel_spmd
    if profile_dir is not None:
        import types, sys
        from trn_agent_boot.trn_boot import _ntff_profile_via_ctypes
        hook = _ntff_profile_via_ctypes("/opt/axon/libaxon_pjrt.so")
        with hook(profile_dir, trace_cores or [0]):
            res = run_bass_kernel_spmd(nc, maps, core_ids=list(range(8)))
    else:
        res = run_bass_kernel_spmd(nc, maps, core_ids=list(range(8)))
    return _combine(res.results, inputs)


def kernel(**inputs):
    return _run(inputs)


# revision 21
# speedup vs baseline: 1.1385x; 1.1385x over previous
# Trainium2 Bass kernel for CausalSelfAttention (B=2, T=2048, C=1024, H=16, D=64)
# with periodic mask: causal AND (key_col % 4 != 3).
#
# Sharding (8 NeuronCores): core c = (b, g) with b = c//4 (batch), g = c%4
# (head group of 4 heads). Each core computes QKV for its 4 heads, attention,
# and a partial output projection y_heads @ Wp[rows] in bf16. Host sums the 4
# partials per batch in fp32 and adds bp (tensor-parallel reduce).
#
# Key device-side choices:
#  - x arrives pre-transposed per batch (xT = x[b].T, [C, T]) so the
#    contraction dim C sits on SBUF partitions for all QKV matmuls.
#  - The periodic mask is exploited as compaction: keys at t%4==3 are never
#    attended; K^T is computed directly at the 1536 kept positions (the K
#    matmul's moving operand skips dropped columns), V is compacted with
#    0/1-selection matmuls.
#  - Scores are produced transposed (S^T[tk_kept, tq]) so softmax-normalized
#    probabilities feed the AV matmul directly as the moving operand; head
#    pairs co-execute on the PE via row groups 0/64.
#  - Boundary (causal-frontier) tiles skip the fully-masked column range both
#    in the scores/exp and in the AV accumulation (no memsets needed).
#  - Softmax row sums ride along the AV matmul via a 64-wide ones block in
#    the stationary operand; reciprocal = exp(-ln(s)) on ScalarE, batched per
#    (head-pair, window) over both heads at once.
#  - Everything runs bf16 on the PE (fp32 PSUM accumulate): fp32 matmuls
#    would run at half rate (fp32_mode=HIGH).
#  - ~40 warm-up matmuls at t=0 lift the PE HAM clock gate from 1.2 GHz to
#    2.4 GHz while the first input DMAs are in flight.
#  - Emission interleaves QKV(j+1) and outproj(j-1) work items between
#    attention chunk-visits of window j so the PE stays fed while ScalarE
#    works through the exp stream.

import ml_dtypes
import numpy as np

B, T, C, H, D = 2, 2048, 1024, 16, 64
HG = 4          # heads per core
CG = HG * D     # = 256 columns of C per core
TK = (T // 4) * 3   # 1536 kept key positions
NTK = TK // 128     # 12 kept-key chunks of 128
SCALE = 1.0 / 8.0   # 1/sqrt(D)
NWARM = 10          # PE warm-up matmuls

_CACHE = {}


def _split_multi_waits(nc, mybir):
    # The pinned walrus here encodes at most 1 sync-wait per instruction
    # (2 for EventSemaphore). Hoist excess waits onto standalone NoOps that
    # precede the instruction on the same engine.
    f = nc.m.functions[0]
    n = 0
    for b in f.blocks:
        insts = list(b.instructions)
        out = []
        changed = False
        for inst in insts:
            si = inst.sync_info
            if si is not None:
                waits = list(si.on_wait)
                cap = 2 if isinstance(inst, mybir.InstEventSemaphore) else 1
                if len(waits) > cap:
                    for w in waits[cap:]:
                        out.append(mybir.InstNoOp(
                            name=f"{inst.name}-ws{n}", engine=inst.engine,
                            ins=[], outs=[],
                            sync_info=mybir.SyncInfo(on_wait=[w], on_update=[])))
                        n += 1
                    inst.sync_info = mybir.SyncInfo(
                        on_wait=waits[:cap], on_update=list(si.on_update))
                    changed = True
            out.append(inst)
        if changed:
            b.instructions = out
    return n


def _build_bass(split=True):
    import concourse.bass as bass
    import concourse.tile as tile
    import concourse.mybir as mybir

    f32 = mybir.dt.float32
    bf16 = mybir.dt.bfloat16

    nc = bass.Bass("TRN2", debug=False, num_devices=8)

    xt_d = nc.dram_tensor("xt", [C, T], bf16, kind="ExternalInput").ap()
    wq_d = nc.dram_tensor("wq", [C, CG], bf16, kind="ExternalInput").ap()
    wk_d = nc.dram_tensor("wk", [C, CG], bf16, kind="ExternalInput").ap()
    wv_d = nc.dram_tensor("wv", [C, CG], bf16, kind="ExternalInput").ap()
    wp_d = nc.dram_tensor("wp", [CG, C], bf16, kind="ExternalInput").ap()
    bq_d = nc.dram_tensor("bq2", [128, 2], f32, kind="ExternalInput").ap()
    bk_d = nc.dram_tensor("bk2", [128, 2], f32, kind="ExternalInput").ap()
    bvb_d = nc.dram_tensor("bvb", [128, HG, D], f32, kind="ExternalInput").ap()
    cm_d = nc.dram_tensor("cmask", [128, 3, 512], bf16, kind="ExternalInput").ap()
    gs_d = nc.dram_tensor("gsel", [128, 6, 128], bf16, kind="ExternalInput").ap()
    out_d = nc.dram_tensor("out", [T, C], bf16, kind="ExternalOutput").ap()

    Exp = mybir.ActivationFunctionType.Exp
    Ln = mybir.ActivationFunctionType.Ln
    MULT = mybir.AluOpType.mult

    with tile.TileContext(nc) as tc, \
         tc.tile_pool(name="persist", bufs=1) as persist, \
         tc.tile_pool(name="work", bufs=1) as work, \
         tc.tile_pool(name="ps_acc", space="PSUM", bufs=2) as ps_acc, \
         tc.tile_pool(name="ps_sc", space="PSUM", bufs=2) as ps_sc, \
         tc.tile_pool(name="ps_y", space="PSUM", bufs=1) as ps_y:
        # ---------- persistent SBUF ----------
        qt = [persist.tile([128, T], bf16, name=f"qt{m}", tag=f"qt{m}") for m in range(2)]
        kt = [persist.tile([128, TK], bf16, name=f"kt{m}", tag=f"kt{m}") for m in range(2)]
        vsb = persist.tile([128, NTK, HG, 2 * D], bf16, name="vsb", tag="vsb")
        yt = [persist.tile([128, T], bf16, name=f"yt{m}", tag=f"yt{m}") for m in range(2)]
        cmask = persist.tile([128, 3, 512], bf16, name="cmask", tag="cmask")
        bqs = persist.tile([128, 2], f32, name="bqs", tag="bqs")
        bks = persist.tile([128, 2], f32, name="bks", tag="bks")
        bvb = persist.tile([128, HG, D], f32, name="bvb", tag="bvb")
        bvf = bvb[:].rearrange("p h d -> p (h d)")
        wp_t = persist.tile([128, 2, C], bf16, name="wp_t", tag="wp_t")
        gsel = persist.tile([128, 6, 128], bf16, name="gsel", tag="gsel")
        wq_t = persist.tile([128, 8, CG], bf16, name="wq_t", tag="wq_t")
        wk_t = persist.tile([128, 8, CG], bf16, name="wk_t", tag="wk_t")
        wv_t = persist.tile([128, 8, CG], bf16, name="wv_t", tag="wv_t")
        warm = persist.tile([128, 512], bf16, name="warm", tag="warm")

        # ---------- DMA plan ----------
        # gpsimd: memsets first (warm-up + ones block), then SWDGE for
        # constants and wp (keeps both HWDGE rings free for wq/wk/wv/x).
        nc.gpsimd.memset(warm[:], 0.0)
        nc.gpsimd.memset(vsb[:, :, :, D:2 * D], 1.0)
        nc.gpsimd.dma_start(bqs[:], bq_d[:])
        nc.gpsimd.dma_start(bks[:], bk_d[:])
        nc.gpsimd.dma_start(bvb[:], bvb_d[:])
        nc.gpsimd.dma_start(wp_t[:], wp_d.rearrange("(k p) n -> p k n", p=128))

        xr = xt_d.rearrange("(k p) t -> p k t", p=128)
        xt = []   # [j] -> [128, 8, 512] column window (k on the free axis)
        for j in range(4):
            xw = work.tile([128, 8, 512], bf16, name=f"x{j}", tag=f"xtw{j % 2}",
                           bufs=2)
            xt.append(xw)

        # sync HWDGE ring: wq halves and first x half interleaved so the
        # first QKV accumulation chain can start as early as possible.
        wqr = wq_d.rearrange("(k p) n -> p k n", p=128)
        nc.sync.dma_start(wq_t[:, 0:4, :], wqr[:, 0:4, :])
        nc.sync.dma_start(xt[0][:, 0:4, :], xr[:, 0:4, 0:512])
        nc.sync.dma_start(wq_t[:, 4:8, :], wqr[:, 4:8, :])
        # scalar HWDGE ring: second half of window 0 first, then wk/wv +
        # small bf16 constants. Scalar engine is idle until the first exp.
        nc.scalar.dma_start(xt[0][:, 4:8, :], xr[:, 4:8, 0:512])
        nc.scalar.dma_start(wk_t[:], wk_d.rearrange("(k p) n -> p k n", p=128))
        nc.scalar.dma_start(wv_t[:, 0:4, :],
                            wv_d.rearrange("(k p) n -> p k n", p=128)[:, 0:4, :])
        nc.scalar.dma_start(wv_t[:, 4:8, :],
                            wv_d.rearrange("(k p) n -> p k n", p=128)[:, 4:8, :])
        nc.scalar.dma_start(cmask[:], cm_d[:])
        nc.scalar.dma_start(gsel[:], gs_d[:])
        # remaining x windows on the sync ring
        for j in range(1, 4):
            for k4 in range(2):
                nc.sync.dma_start(xt[j][:, 4 * k4:4 * k4 + 4, :],
                                  xr[:, 4 * k4:4 * k4 + 4, 512 * j:512 * (j + 1)])

        # ---------- PE warm-up (lift HAM to 2.4 GHz during input DMA) ----------
        pw = ps_sc.tile([128, 2, 512], f32, tag="acc")
        for _ in range(NWARM):
            nc.tensor.matmul(pw[:, 0, :], warm[:, 0:128], warm[:],
                             start=True, stop=True)

        # ---------- work items ----------
        def q_item(j, m):
            def emit():
                pq = ps_acc.tile([128, 512], f32, tag="acc")
                for k in range(8):
                    nc.tensor.matmul(pq[:], wq_t[:, k, 128 * m:128 * (m + 1)],
                                     xt[j][:, k, :], start=(k == 0), stop=(k == 7))
                nc.vector.tensor_scalar_add(qt[m][:, 512 * j:512 * (j + 1)],
                                            pq[:], bqs[:, m:m + 1])
            return emit

        def k_item(j, m):
            def emit():
                pk = ps_acc.tile([128, 512], f32, tag="acc")
                for k in range(8):
                    nc.tensor.matmul(pk[:], wk_t[:, k, 128 * m:128 * (m + 1)],
                                     xt[j][:, k, :], start=(k == 0), stop=(k == 7))
                # compact to kept key columns (drop t%4==3) during the copy
                pkc = pk[:].rearrange("p (a b) -> p a b", b=4)[:, :, 0:3]
                nc.vector.tensor_scalar_add(kt[m][:, 384 * j:384 * (j + 1)],
                                            pkc, bks[:, m:m + 1])
            return emit

        vfull = {}
        def v_item(j, mm):
            def emit():
                pv = ps_acc.tile([128, 512], f32, tag="acc")
                for k in range(8):
                    nc.tensor.matmul(pv[:, 0:CG],
                                     xt[j][:, k, 128 * mm:128 * (mm + 1)],
                                     wv_t[:, k, :], start=(k == 0), stop=(k == 7))
                vf = work.tile([128, CG], bf16, name=f"vf{j}_{mm}", tag="vf",
                               bufs=6)
                nc.vector.scalar_tensor_tensor(
                    out=vf[:], in0=pv[:, 0:CG], scalar=1.0, in1=bvf[:],
                    op0=mybir.AluOpType.bypass, op1=mybir.AluOpType.add)
                vfull[(j, mm)] = vf
            return emit

        def g_item(j, s):
            def emit():
                i = 3 * j + s
                pvk = ps_acc.tile([128, 512], f32, tag="acc")
                nc.tensor.matmul(pvk[:, 0:CG], gsel[:, 2 * s, :],
                                 vfull[(j, s)][:], start=True, stop=False)
                nc.tensor.matmul(pvk[:, 0:CG], gsel[:, 2 * s + 1, :],
                                 vfull[(j, s + 1)][:], start=False, stop=True)
                nc.vector.tensor_copy(
                    vsb[:, i, :, 0:D],
                    pvk[:, 0:CG].rearrange("p (h d) -> p h d", d=D))
            return emit

        def qkv_items(j):
            # gathers interleaved behind the V chains they depend on, so the
            # short gather matmul pairs overlap the long V accumulations
            return ([q_item(j, m) for m in range(2)]
                    + [k_item(j, m) for m in range(2)]
                    + [v_item(j, 0), v_item(j, 1), v_item(j, 2), g_item(j, 0),
                       v_item(j, 3), g_item(j, 1), g_item(j, 2)])

        def outproj_item(m):
            def emit():
                stage = work.tile([128, C], bf16, tag="stage", bufs=2)
                for n in range(2):
                    po = ps_acc.tile([128, 512], f32, tag="acc")
                    for k2 in range(2):
                        nc.tensor.matmul(
                            po[:], yt[k2][:, 128 * m:128 * (m + 1)],
                            wp_t[:, k2, 512 * n:512 * (n + 1)],
                            start=(k2 == 0), stop=(k2 == 1))
                    nc.vector.tensor_copy(stage[:, 512 * n:512 * (n + 1)], po[:])
                nc.sync.dma_start(out_d[128 * m:128 * (m + 1), :], stage[:])
            return emit

        # ---------- QKV window 0 (nothing to interleave with) ----------
        for it in qkv_items(0):
            it()

        # ---------- attention windows with interleaved fillers ----------
        for j in range(4):
            jwin = slice(512 * j, 512 * (j + 1))
            ntile = 3 * (j + 1)
            nb0 = ntile - 3  # first boundary tile index
            fillers = []
            if j < 3:
                fillers += qkv_items(j + 1)
            if j >= 1:
                fillers += [outproj_item(m) for m in range(4 * (j - 1), 4 * j)]
            nvisit = 2 * ntile
            nfill = len(fillers)
            vdone = 0
            fdone = 0

            for hp in range(2):
                pys = ps_y.tile([128, 2, 512], f32, tag="py")
                for i in range(ntile):
                    u = i - nb0
                    # boundary tiles u=1,2: cols [0:off) are fully masked —
                    # skipped in scores, exp, and the AV accumulation.
                    off = (0, 128, 320)[u] if u >= 1 else 0
                    ps2 = ps_sc.tile([128, 2, 512], f32, tag="acc")
                    pt2 = work.tile([128, 2, 512], bf16, tag="pt2", bufs=4)
                    for q in range(2):  # q: row group (head 2*hp + q)
                        nc.tensor.matmul(
                            ps2[:, q, off:512],
                            kt[hp][64 * q:64 * q + 64, 128 * i:128 * (i + 1)],
                            qt[hp][64 * q:64 * q + 64,
                                   512 * j + off:512 * (j + 1)],
                            start=True, stop=True)
                    nc.scalar.activation(pt2[:, :, off:512], ps2[:, :, off:512],
                                         Exp, bias=0.0, scale=SCALE)
                    if u >= 0:  # boundary tile: causal mask (both heads)
                        w = (192, 384, 512)[u]
                        for q in range(2):
                            nc.vector.tensor_tensor(
                                pt2[:, q, off:w], pt2[:, q, off:w],
                                cmask[:, u, off:w], op=MULT)
                    for q in range(2):
                        nc.tensor.matmul(
                            pys[:, q, off:512], vsb[:, i, 2 * hp + q, :],
                            pt2[:, q, off:512],
                            start=(i == 0), stop=(i == ntile - 1))
                    # interleave filler work to keep the PE fed while the
                    # scalar engine chews through the exp stream
                    vdone += 1
                    while fillers and fdone < (nfill * vdone) // nvisit:
                        fillers.pop(0)()
                        fdone += 1

                # a filler around the row boundary covers the normalize chain
                if fillers:
                    fillers.pop(0)()
                    fdone += 1
                # softmax normalization for both heads of this pair at once:
                # rec = exp(-ln(rowsum)), rowsums replicated at rows 64:128
                lns = work.tile([64, 2, 512], f32, tag="lns", bufs=2)
                rec = work.tile([64, 2, 512], f32, tag="rec", bufs=2)
                nc.scalar.activation(lns[:], pys[64:128, :, :], Ln)
                nc.scalar.activation(rec[:], lns[:], Exp, bias=0.0, scale=-1.0)
                for q in range(2):
                    nc.vector.tensor_tensor(
                        yt[hp][64 * q:64 * q + 64, jwin],
                        pys[0:64, q, :], rec[:, q, :], op=MULT)

            # any leftover fillers for this window
            for it in fillers:
                it()

        # ---------- output projection for the last window ----------
        for m in range(12, 16):
            outproj_item(m)()

    if split:
        _split_multi_waits(nc, mybir)
    return nc


def _get_nc():
    if "nc" not in _CACHE:
        _CACHE["nc"] = _build_bass()
    return _CACHE["nc"]


def _host_maps(inputs):
    x = np.asarray(inputs["x"], np.float32)
    Wq = np.asarray(inputs["Wq"], np.float32)
    Wk = np.asarray(inputs["Wk"], np.float32)
    Wv = np.asarray(inputs["Wv"], np.float32)
    Wp = np.asarray(inputs["Wp"], np.float32)
    bq = np.asarray(inputs["bq"], np.float32)
    bk = np.asarray(inputs["bk"], np.float32)
    bv = np.asarray(inputs["bv"], np.float32)

    # causal masks in compacted key coordinates: 3 boundary chunks
    p = np.arange(128)
    f = np.arange(512)
    cm = np.zeros((128, 3, 512), np.float32)
    for u in range(3):
        q = 128 * u + p
        g = (q // 3) * 4 + (q % 3)
        cm[:, u, :] = (f[None, :] >= g[:, None]).astype(np.float32)

    # V row-gather selection matrices: kept chunk i = 3k+s draws rows from
    # original chunks 4k+s and 4k+s+1; G[s][side][p, m] = 1 iff kept row m
    # maps to row p of that original chunk.
    gs = np.zeros((128, 6, 128), np.float32)
    for s in range(3):
        for m in range(128):
            orr = ((128 * s + m) // 3) * 4 + (128 * s + m) % 3
            side = 0 if orr < 128 * (s + 1) else 1
            gs[orr - 128 * (s + side), 2 * s + side, m] = 1.0

    xts = [np.ascontiguousarray(x[b].T).astype(ml_dtypes.bfloat16) for b in range(B)]
    maps = []
    for c in range(8):
        b, g = c // 4, c % 4
        sl = slice(CG * g, CG * (g + 1))
        maps.append({
            "xt": xts[b],
            "wq": np.ascontiguousarray(Wq[:, sl]).astype(ml_dtypes.bfloat16),
            "wk": np.ascontiguousarray(Wk[:, sl]).astype(ml_dtypes.bfloat16),
            "wv": np.ascontiguousarray(Wv[:, sl]).astype(ml_dtypes.bfloat16),
            "wp": np.ascontiguousarray(Wp[sl, :]).astype(ml_dtypes.bfloat16),
            "bq2": np.ascontiguousarray(bq[sl].reshape(2, 128).T),
            "bk2": np.ascontiguousarray(bk[sl].reshape(2, 128).T),
            "bvb": np.ascontiguousarray(
                np.broadcast_to(bv[sl].reshape(HG, D), (128, HG, D))),
            "cmask": cm.astype(ml_dtypes.bfloat16),
            "gsel": gs.astype(ml_dtypes.bfloat16),
        })
    return maps


def _combine(results, inputs):
    bp = np.asarray(inputs["bp"], np.float32)
    out = np.zeros((B, T, C), np.float32)
    for c in range(8):
        out[c // 4] += np.asarray(results[c]["out"], dtype=np.float32)
    out += bp[None, None, :]
    return out


def _run(inputs, profile_dir=None, trace_cores=None):
    nc = _get_nc()
    maps = _host_maps(inputs)
    from concourse.bass_utils import run_bass_kernel_spmd
    if profile_dir is not None:
        import types, sys
        from trn_agent_boot.trn_boot import _ntff_profile_via_ctypes
        hook = _ntff_profile_via_ctypes("/opt/axon/libaxon_pjrt.so")
        with hook(profile_dir, trace_cores or [0]):
            res = run_bass_kernel_spmd(nc, maps, core_ids=list(range(8)))
    else:
        res = run_bass_kernel_spmd(nc, maps, core_ids=list(range(8)))
    return _combine(res.results, inputs)


def kernel(**inputs):
    return _run(inputs)


# revision 24
# speedup vs baseline: 1.1832x; 1.0393x over previous
# Trainium2 Bass kernel for CausalSelfAttention (B=2, T=2048, C=1024, H=16, D=64)
# with periodic mask: causal AND (key_col % 4 != 3).
#
# Sharding (8 NeuronCores): core c = (b, g) with b = c//4 (batch), g = c%4
# (head group of 4 heads). Each core computes QKV for its 4 heads, attention,
# and a partial output projection y_heads @ Wp[rows] in bf16. Host sums the 4
# partials per batch in fp32 and adds bp (tensor-parallel reduce).
#
# Key device-side choices:
#  - x arrives pre-transposed per batch (xT = x[b].T, [C, T]) so the
#    contraction dim C sits on SBUF partitions for all QKV matmuls.
#  - The periodic mask is exploited as compaction: keys at t%4==3 are never
#    attended; K^T is computed directly at the 1536 kept positions (the K
#    matmul's moving operand skips dropped columns), V is compacted with
#    0/1-selection matmuls.
#  - Scores are produced transposed (S^T[tk_kept, tq]) so softmax-normalized
#    probabilities feed the AV matmul directly as the moving operand; head
#    pairs co-execute on the PE via row groups 0/64.
#  - Boundary (causal-frontier) tiles skip the fully-masked column range both
#    in the scores/exp and in the AV accumulation (no memsets needed).
#  - Softmax row sums ride along the AV matmul via a 64-wide ones block in
#    the stationary operand; reciprocal = exp(-ln(s)) on ScalarE, batched per
#    (head-pair, window) over both heads at once.
#  - Everything runs bf16 on the PE (fp32 PSUM accumulate): fp32 matmuls
#    would run at half rate (fp32_mode=HIGH).
#  - ~40 warm-up matmuls at t=0 lift the PE HAM clock gate from 1.2 GHz to
#    2.4 GHz while the first input DMAs are in flight.
#  - Emission interleaves QKV(j+1) and outproj(j-1) work items between
#    attention chunk-visits of window j so the PE stays fed while ScalarE
#    works through the exp stream.

import ml_dtypes
import numpy as np

B, T, C, H, D = 2, 2048, 1024, 16, 64
HG = 4          # heads per core
CG = HG * D     # = 256 columns of C per core
TK = (T // 4) * 3   # 1536 kept key positions
NTK = TK // 128     # 12 kept-key chunks of 128
SCALE = 1.0 / 8.0   # 1/sqrt(D)
NWARM = 10          # PE warm-up matmuls

_CACHE = {}


def _split_multi_waits(nc, mybir):
    # The pinned walrus here encodes at most 1 sync-wait per instruction
    # (2 for EventSemaphore). Hoist excess waits onto standalone NoOps that
    # precede the instruction on the same engine.
    f = nc.m.functions[0]
    n = 0
    for b in f.blocks:
        insts = list(b.instructions)
        out = []
        changed = False
        for inst in insts:
            si = inst.sync_info
            if si is not None:
                waits = list(si.on_wait)
                cap = 2 if isinstance(inst, mybir.InstEventSemaphore) else 1
                if len(waits) > cap:
                    for w in waits[cap:]:
                        out.append(mybir.InstNoOp(
                            name=f"{inst.name}-ws{n}", engine=inst.engine,
                            ins=[], outs=[],
                            sync_info=mybir.SyncInfo(on_wait=[w], on_update=[])))
                        n += 1
                    inst.sync_info = mybir.SyncInfo(
                        on_wait=waits[:cap], on_update=list(si.on_update))
                    changed = True
            out.append(inst)
        if changed:
            b.instructions = out
    return n


def _build_bass(split=True):
    import concourse.bass as bass
    import concourse.tile as tile
    import concourse.mybir as mybir

    f32 = mybir.dt.float32
    bf16 = mybir.dt.bfloat16

    nc = bass.Bass("TRN2", debug=False, num_devices=8)

    xt_d = nc.dram_tensor("xt", [C, T], bf16, kind="ExternalInput").ap()
    wq_d = nc.dram_tensor("wq", [C, CG], bf16, kind="ExternalInput").ap()
    wk_d = nc.dram_tensor("wk", [C, CG], bf16, kind="ExternalInput").ap()
    wv_d = nc.dram_tensor("wv", [C, CG], bf16, kind="ExternalInput").ap()
    wp_d = nc.dram_tensor("wp", [CG, C], bf16, kind="ExternalInput").ap()
    bq_d = nc.dram_tensor("bq2", [128, 2], f32, kind="ExternalInput").ap()
    bk_d = nc.dram_tensor("bk2", [128, 2], f32, kind="ExternalInput").ap()
    bvb_d = nc.dram_tensor("bvb", [128, HG, D], f32, kind="ExternalInput").ap()
    cm_d = nc.dram_tensor("cmask", [128, 3, 512], bf16, kind="ExternalInput").ap()
    gs_d = nc.dram_tensor("gsel", [128, 6, 128], bf16, kind="ExternalInput").ap()
    out_d = nc.dram_tensor("out", [T, C], bf16, kind="ExternalOutput").ap()

    Exp = mybir.ActivationFunctionType.Exp
    Ln = mybir.ActivationFunctionType.Ln
    MULT = mybir.AluOpType.mult

    with tile.TileContext(nc) as tc, \
         tc.tile_pool(name="persist", bufs=1) as persist, \
         tc.tile_pool(name="work", bufs=1) as work, \
         tc.tile_pool(name="ps_acc", space="PSUM", bufs=2) as ps_acc, \
         tc.tile_pool(name="ps_sc", space="PSUM", bufs=2) as ps_sc, \
         tc.tile_pool(name="ps_y", space="PSUM", bufs=1) as ps_y:
        # ---------- persistent SBUF ----------
        qt = [persist.tile([128, T], bf16, name=f"qt{m}", tag=f"qt{m}") for m in range(2)]
        kt = [persist.tile([128, TK], bf16, name=f"kt{m}", tag=f"kt{m}") for m in range(2)]
        vsb = persist.tile([128, NTK, HG, 2 * D], bf16, name="vsb", tag="vsb")
        yt = [persist.tile([128, T], bf16, name=f"yt{m}", tag=f"yt{m}") for m in range(2)]
        cmask = persist.tile([128, 3, 512], bf16, name="cmask", tag="cmask")
        bqs = persist.tile([128, 2], f32, name="bqs", tag="bqs")
        bks = persist.tile([128, 2], f32, name="bks", tag="bks")
        bvb = persist.tile([128, HG, D], f32, name="bvb", tag="bvb")
        bvf = bvb[:].rearrange("p h d -> p (h d)")
        wp_t = persist.tile([128, 2, C], bf16, name="wp_t", tag="wp_t")
        gsel = persist.tile([128, 6, 128], bf16, name="gsel", tag="gsel")
        wq_t = persist.tile([128, 8, CG], bf16, name="wq_t", tag="wq_t")
        wk_t = persist.tile([128, 8, CG], bf16, name="wk_t", tag="wk_t")
        wv_t = persist.tile([128, 8, CG], bf16, name="wv_t", tag="wv_t")
        warm = persist.tile([128, 512], bf16, name="warm", tag="warm")

        # ---------- DMA plan ----------
        # gpsimd: memsets first (warm-up + ones block), then SWDGE for
        # constants and wp (keeps both HWDGE rings free for wq/wk/wv/x).
        nc.gpsimd.memset(warm[:], 0.0)
        nc.gpsimd.memset(vsb[:, :, :, D:2 * D], 1.0)
        nc.gpsimd.dma_start(bqs[:], bq_d[:])
        nc.gpsimd.dma_start(bks[:], bk_d[:])
        nc.gpsimd.dma_start(bvb[:], bvb_d[:])
        nc.gpsimd.dma_start(wp_t[:], wp_d.rearrange("(k p) n -> p k n", p=128))

        xr = xt_d.rearrange("(k p) t -> p k t", p=128)
        xt = []   # [j] -> [128, 8, 512] column window (k on the free axis)
        for j in range(4):
            xw = work.tile([128, 8, 512], bf16, name=f"x{j}", tag=f"xtw{j % 2}",
                           bufs=2)
            xt.append(xw)

        # sync HWDGE ring: wq halves and first x half interleaved so the
        # first QKV accumulation chain can start as early as possible.
        wqr = wq_d.rearrange("(k p) n -> p k n", p=128)
        nc.sync.dma_start(wq_t[:, 0:4, :], wqr[:, 0:4, :])
        nc.sync.dma_start(xt[0][:, 0:4, :], xr[:, 0:4, 0:512])
        nc.sync.dma_start(wq_t[:, 4:8, :], wqr[:, 4:8, :])
        # scalar HWDGE ring: second half of window 0 first, then wk/wv +
        # small bf16 constants. Scalar engine is idle until the first exp.
        nc.scalar.dma_start(xt[0][:, 4:8, :], xr[:, 4:8, 0:512])
        nc.scalar.dma_start(wk_t[:], wk_d.rearrange("(k p) n -> p k n", p=128))
        nc.scalar.dma_start(wv_t[:, 0:4, :],
                            wv_d.rearrange("(k p) n -> p k n", p=128)[:, 0:4, :])
        nc.scalar.dma_start(wv_t[:, 4:8, :],
                            wv_d.rearrange("(k p) n -> p k n", p=128)[:, 4:8, :])
        nc.scalar.dma_start(cmask[:], cm_d[:])
        nc.scalar.dma_start(gsel[:], gs_d[:])
        # remaining x windows on the sync ring
        for j in range(1, 4):
            for k4 in range(2):
                nc.sync.dma_start(xt[j][:, 4 * k4:4 * k4 + 4, :],
                                  xr[:, 4 * k4:4 * k4 + 4, 512 * j:512 * (j + 1)])

        # ---------- PE warm-up (lift HAM to 2.4 GHz during input DMA) ----------
        pw = ps_sc.tile([128, 2, 512], f32, tag="acc")
        for _ in range(NWARM):
            nc.tensor.matmul(pw[:, 0, :], warm[:, 0:128], warm[:],
                             start=True, stop=True)

        # ---------- work items ----------
        def q_item(j, m):
            def emit():
                pq = ps_acc.tile([128, 512], f32, tag="acc")
                for k in range(8):
                    nc.tensor.matmul(pq[:], wq_t[:, k, 128 * m:128 * (m + 1)],
                                     xt[j][:, k, :], start=(k == 0), stop=(k == 7))
                nc.vector.tensor_scalar_add(qt[m][:, 512 * j:512 * (j + 1)],
                                            pq[:], bqs[:, m:m + 1])
            return emit

        def k_item(j, m):
            def emit():
                pk = ps_acc.tile([128, 512], f32, tag="acc")
                for k in range(8):
                    nc.tensor.matmul(pk[:], wk_t[:, k, 128 * m:128 * (m + 1)],
                                     xt[j][:, k, :], start=(k == 0), stop=(k == 7))
                # compact to kept key columns (drop t%4==3) during the copy
                pkc = pk[:].rearrange("p (a b) -> p a b", b=4)[:, :, 0:3]
                nc.vector.tensor_scalar_add(kt[m][:, 384 * j:384 * (j + 1)],
                                            pkc, bks[:, m:m + 1])
            return emit

        vfull = {}
        def v_item(j, mm):
            def emit():
                pv = ps_acc.tile([128, 512], f32, tag="acc")
                for k in range(8):
                    nc.tensor.matmul(pv[:, 0:CG],
                                     xt[j][:, k, 128 * mm:128 * (mm + 1)],
                                     wv_t[:, k, :], start=(k == 0), stop=(k == 7))
                vf = work.tile([128, CG], bf16, name=f"vf{j}_{mm}", tag="vf",
                               bufs=6)
                nc.vector.scalar_tensor_tensor(
                    out=vf[:], in0=pv[:, 0:CG], scalar=1.0, in1=bvf[:],
                    op0=mybir.AluOpType.bypass, op1=mybir.AluOpType.add)
                vfull[(j, mm)] = vf
            return emit

        def g_item(j, s):
            def emit():
                i = 3 * j + s
                pvk = ps_acc.tile([128, 512], f32, tag="acc")
                nc.tensor.matmul(pvk[:, 0:CG], gsel[:, 2 * s, :],
                                 vfull[(j, s)][:], start=True, stop=False)
                nc.tensor.matmul(pvk[:, 0:CG], gsel[:, 2 * s + 1, :],
                                 vfull[(j, s + 1)][:], start=False, stop=True)
                nc.vector.tensor_copy(
                    vsb[:, i, :, 0:D],
                    pvk[:, 0:CG].rearrange("p (h d) -> p h d", d=D))
            return emit

        def qkv_items(j):
            # gathers interleaved behind the V chains they depend on, so the
            # short gather matmul pairs overlap the long V accumulations
            return ([q_item(j, m) for m in range(2)]
                    + [k_item(j, m) for m in range(2)]
                    + [v_item(j, 0), v_item(j, 1), v_item(j, 2), g_item(j, 0),
                       v_item(j, 3), g_item(j, 1), g_item(j, 2)])

        def outproj_item(m):
            def emit():
                stage = work.tile([128, C], bf16, tag="stage", bufs=2)
                for n in range(2):
                    po = ps_acc.tile([128, 512], f32, tag="acc")
                    for k2 in range(2):
                        nc.tensor.matmul(
                            po[:], yt[k2][:, 128 * m:128 * (m + 1)],
                            wp_t[:, k2, 512 * n:512 * (n + 1)],
                            start=(k2 == 0), stop=(k2 == 1))
                    nc.vector.tensor_copy(stage[:, 512 * n:512 * (n + 1)], po[:])
                nc.sync.dma_start(out_d[128 * m:128 * (m + 1), :], stage[:])
            return emit

        # ---------- QKV window 0 (nothing to interleave with) ----------
        for it in qkv_items(0):
            it()

        # ---------- attention windows with interleaved fillers ----------
        for j in range(4):
            jwin = slice(512 * j, 512 * (j + 1))
            ntile = 3 * (j + 1)
            nb0 = ntile - 3  # first boundary tile index
            fillers = []
            if j < 3:
                fillers += qkv_items(j + 1)
            if j >= 1:
                fillers += [outproj_item(m) for m in range(4 * (j - 1), 4 * j)]
            nvisit = 2 * ntile
            nfill = len(fillers)
            vdone = 0
            fdone = 0

            for hp in range(2):
                pys = ps_y.tile([128, 2, 512], f32, tag="py")
                for i in range(ntile):
                    u = i - nb0
                    # boundary tiles u=1,2: cols [0:off) are fully masked —
                    # skipped in scores, exp, and the AV accumulation.
                    off = (0, 128, 320)[u] if u >= 1 else 0
                    ps2 = ps_sc.tile([128, 2, 512], f32, tag="acc")
                    pt2 = work.tile([128, 2, 512], bf16, tag="pt2", bufs=4)
                    for q in range(2):  # q: row group (head 2*hp + q)
                        nc.tensor.matmul(
                            ps2[:, q, off:512],
                            kt[hp][64 * q:64 * q + 64, 128 * i:128 * (i + 1)],
                            qt[hp][64 * q:64 * q + 64,
                                   512 * j + off:512 * (j + 1)],
                            start=True, stop=True)
                    nc.scalar.activation(pt2[:, :, off:512], ps2[:, :, off:512],
                                         Exp, bias=0.0, scale=SCALE)
                    if u >= 0:  # boundary tile: causal mask (both heads)
                        w = (192, 384, 512)[u]
                        for q in range(2):
                            nc.vector.tensor_tensor(
                                pt2[:, q, off:w], pt2[:, q, off:w],
                                cmask[:, u, off:w], op=MULT)
                    for q in range(2):
                        nc.tensor.matmul(
                            pys[:, q, off:512], vsb[:, i, 2 * hp + q, :],
                            pt2[:, q, off:512],
                            start=(i == 0), stop=(i == ntile - 1))
                    # interleave filler work to keep the PE fed while the
                    # scalar engine chews through the exp stream; hold two
                    # items back to cover the hp0->hp1 normalize latency
                    vdone += 1
                    while (len(fillers) > (2 if hp == 0 else 0)
                           and fdone < (nfill * vdone) // nvisit):
                        fillers.pop(0)()
                        fdone += 1

                # softmax normalization for both heads of this pair at once:
                # rec = exp(-ln(rowsum)), rowsums replicated at rows 64:128
                lns = work.tile([64, 2, 512], f32, tag="lns", bufs=2)
                rec = work.tile([64, 2, 512], f32, tag="rec", bufs=2)
                nc.scalar.activation(lns[:], pys[64:128, :, :], Ln)
                nc.scalar.activation(rec[:], lns[:], Exp, bias=0.0, scale=-1.0)
                for q in range(2):
                    nc.vector.tensor_tensor(
                        yt[hp][64 * q:64 * q + 64, jwin],
                        pys[0:64, q, :], rec[:, q, :], op=MULT)
                if hp == 0:
                    # reserved fillers execute during the normalize chain so
                    # the PE isn't idle before hp1's AV can start
                    for _ in range(2):
                        if fillers:
                            fillers.pop(0)()
                            fdone += 1

            # any leftover fillers for this window
            for it in fillers:
                it()

        # ---------- output projection for the last window ----------
        for m in range(12, 16):
            outproj_item(m)()

    if split:
        _split_multi_waits(nc, mybir)
    return nc


def _get_nc():
    if "nc" not in _CACHE:
        _CACHE["nc"] = _build_bass()
    return _CACHE["nc"]


def _host_maps(inputs):
    x = np.asarray(inputs["x"], np.float32)
    Wq = np.asarray(inputs["Wq"], np.float32)
    Wk = np.asarray(inputs["Wk"], np.float32)
    Wv = np.asarray(inputs["Wv"], np.float32)
    Wp = np.asarray(inputs["Wp"], np.float32)
    bq = np.asarray(inputs["bq"], np.float32)
    bk = np.asarray(inputs["bk"], np.float32)
    bv = np.asarray(inputs["bv"], np.float32)

    # causal masks in compacted key coordinates: 3 boundary chunks
    p = np.arange(128)
    f = np.arange(512)
    cm = np.zeros((128, 3, 512), np.float32)
    for u in range(3):
        q = 128 * u + p
        g = (q // 3) * 4 + (q % 3)
        cm[:, u, :] = (f[None, :] >= g[:, None]).astype(np.float32)

    # V row-gather selection matrices: kept chunk i = 3k+s draws rows from
    # original chunks 4k+s and 4k+s+1; G[s][side][p, m] = 1 iff kept row m
    # maps to row p of that original chunk.
    gs = np.zeros((128, 6, 128), np.float32)
    for s in range(3):
        for m in range(128):
            orr = ((128 * s + m) // 3) * 4 + (128 * s + m) % 3
            side = 0 if orr < 128 * (s + 1) else 1
            gs[orr - 128 * (s + side), 2 * s + side, m] = 1.0

    xts = [np.ascontiguousarray(x[b].T).astype(ml_dtypes.bfloat16) for b in range(B)]
    maps = []
    for c in range(8):
        b, g = c // 4, c % 4
        sl = slice(CG * g, CG * (g + 1))
        maps.append({
            "xt": xts[b],
            "wq": np.ascontiguousarray(Wq[:, sl]).astype(ml_dtypes.bfloat16),
            "wk": np.ascontiguousarray(Wk[:, sl]).astype(ml_dtypes.bfloat16),
            "wv": np.ascontiguousarray(Wv[:, sl]).astype(ml_dtypes.bfloat16),
            "wp": np.ascontiguousarray(Wp[sl, :]).astype(ml_dtypes.bfloat16),
            "bq2": np.ascontiguousarray(bq[sl].reshape(2, 128).T),
            "bk2": np.ascontiguousarray(bk[sl].reshape(2, 128).T),
            "bvb": np.ascontiguousarray(
                np.broadcast_to(bv[sl].reshape(HG, D), (128, HG, D))),
            "cmask": cm.astype(ml_dtypes.bfloat16),
            "gsel": gs.astype(ml_dtypes.bfloat16),
        })
    return maps


def _combine(results, inputs):
    bp = np.asarray(inputs["bp"], np.float32)
    out = np.zeros((B, T, C), np.float32)
    for c in range(8):
        out[c // 4] += np.asarray(results[c]["out"], dtype=np.float32)
    out += bp[None, None, :]
    return out


def _run(inputs, profile_dir=None, trace_cores=None):
    nc = _get_nc()
    maps = _host_maps(inputs)
    from concourse.bass_utils import run_bass_kernel_spmd
    if profile_dir is not None:
        import types, sys
        from trn_agent_boot.trn_boot import _ntff_profile_via_ctypes
        hook = _ntff_profile_via_ctypes("/opt/axon/libaxon_pjrt.so")
        with hook(profile_dir, trace_cores or [0]):
            res = run_bass_kernel_spmd(nc, maps, core_ids=list(range(8)))
    else:
        res = run_bass_kernel_spmd(nc, maps, core_ids=list(range(8)))
    return _combine(res.results, inputs)


def kernel(**inputs):
    return _run(inputs)
